# revision 1
# baseline (speedup 1.0000x reference)
"""AttackNet kernel for 8 Trainium2 NeuronCores.

Reference computation:
    out  = conv1x1(x, W) + b                        # 60 channels
    pert = out.reshape(n, 20, 3, h, w)[arange, target]
    pert = ((pert - min) / (max - min) - 0.5) * 2   # per (sample, channel) spatial
    return pert * (MAX_PERTURBATION / 128)

Only the 3 gathered channels per sample are needed, so the host picks the
per-sample 3x3 weight block W[3t:3t+3, :] and the device computes
    lin[j]  = w0*x0 + w1*x1 + w2*x2                 (per sample)
    out[j]  = (lin - min) * (2 / (max - min)) - 1
The bias cancels exactly inside the min/max normalization and is dropped.

Sharding: pure data parallel, 4 samples per core across 8 cores.

Per-core layout: partition p = sample*32 + spatial_block, free dim 1568
(+pad columns), so one instruction processes all 4 samples of one output
channel j with per-partition scalars carrying per-sample weights.

Per j (3 iterations):
    DVE MULADD2_ATK: q   = x1*w1 + x0*w0                     (custom op)
    DVE LINSTAT_ATK: lin = x2*w2 + q, max -> pad col,
                     min -> accum col                         (custom op)
    PE transpose + DVE group-reduce + PE transpose + tiny ops
        -> per-(sample,j) scale/bias, PE matmul broadcast
    ACT: out = lin*s + t   (activation Identity, per-partition scale/bias)
"""

import sys
import time

sys.path.insert(0, "/opt/trn_rl_repo")
sys.path.insert(0, "/root/problem")

import numpy as np

import concourse.bass as bass  # noqa: F401
import concourse.tile as tile
from concourse import bacc, mybir
from concourse.bass_utils import run_bass_kernel_spmd


def _install_ntff_hook_shim():
    """Provide antenv.axon_hooks (absent in this image) so trace=True works."""
    import types

    if "antenv.axon_hooks" in sys.modules:
        return
    import contextlib
    import ctypes

    so_path = "/opt/axon/libaxon_pjrt.so"
    try:
        lib = ctypes.CDLL(so_path)
        lib.axon_start_nrt_profile.argtypes = [
            ctypes.POINTER(ctypes.c_int64),
            ctypes.c_size_t,
        ]
        lib.axon_start_nrt_profile.restype = ctypes.c_int64
        lib.axon_stop_nrt_profile.argtypes = [ctypes.c_char_p]
        lib.axon_stop_nrt_profile.restype = ctypes.c_int64
    except OSError:
        lib = None

    @contextlib.contextmanager
    def _hook(output_dir, device_ids):
        import jax

        jax.devices()
        if device_ids:
            ids = (ctypes.c_int64 * len(device_ids))(*device_ids)
            rc = lib.axon_start_nrt_profile(ids, len(device_ids))
        else:
            rc = lib.axon_start_nrt_profile(None, 0)
        if rc != 0:
            raise RuntimeError(f"axon_start_nrt_profile rc={rc}")
        try:
            yield
        finally:
            n = lib.axon_stop_nrt_profile(str(output_dir).encode())
            print(f"ntff profile: {n} file(s) written to {output_dir}",
                  file=sys.stderr)

    mod = types.ModuleType("antenv.axon_hooks")
    mod.get_axon_ntff_profile_hook = lambda: (_hook if lib is not None else None)
    mod.set_axon_ntff_profile_hook = lambda h: None
    import antenv

    antenv.axon_hooks = mod
    sys.modules["antenv.axon_hooks"] = mod


_install_ntff_hook_shim()

# registers the custom DVE ops in concourse.dve_ops at import time
try:
    from custom_ops import LINSTAT, MULADD2  # noqa: E402
except ImportError:
    # self-contained fallback when kernel.py is shipped alone
    from concourse import dve_ops
    from concourse.dve_spec import (
        AluOp, C0, C1, C2, Spec, Src0, Src1, lower, scan, select,
    )
    from concourse.dve_uop import DveOpSpec

    def _muladd2_ref(in0, in1, c0, c1, c2):
        return in0 * c0 + in1 * c1

    def _linstat_ref(in0, in1, c0, c1, c2):
        v = (in0 * c0 + in1).astype(np.float32)
        r = np.maximum.accumulate(v, axis=-1)
        o = np.where(in1 <= c1, r, v)
        acc = np.minimum(
            np.float32(c2), o.reshape(o.shape[0], -1).min(-1, keepdims=True)
        )
        return o, acc

    def _register(name, spec):
        for op in dve_ops.OPS:
            if op.name == name:
                return op
        opcode = dve_ops._CUSTOM_DVE_ROW_BASE + len(dve_ops.OPS)
        assert opcode < 0x20
        shas = {}
        for ver in ("v3", "v4"):
            uops = lower(spec, ver=ver)
            shas[ver] = DveOpSpec(
                name=name, opcode=opcode, uops=uops, rd1_en=True
            ).sha(ver)
        op = dve_ops.DveOp(name, spec, subdim=False, uops_sha=shas)
        dve_ops.OPS.append(op)
        dve_ops.CUSTOM_DVE_SPECS[name] = spec
        dve_ops._SUB_OPCODE_FOR_NAME[name] = opcode
        return op

    MULADD2 = _register(
        "MULADD2_ATK", Spec(body=Src0 * C0 + Src1 * C1, reference=_muladd2_ref)
    )
    _v = Src0 * C0 + Src1
    LINSTAT = _register(
        "LINSTAT_ATK",
        Spec(
            body=select(Src1 <= C1, scan(AluOp.MAX, _v), _v),
            accum=AluOp.MIN,
            accum_init=C2,
            reference=_linstat_ref,
        ),
    )

P = 128                 # SBUF partitions
H = W_ = 224
F = H * W_              # 50176 spatial elements per plane
G32 = 32                # partitions per sample group
NS = 4                  # samples per core
FD = F // G32           # 1568 free elements per partition
NCORES = 8
N = NCORES * NS         # 32 samples total
OUT_CHANNELS = 3
MAX_PERTURBATION = 128.0
OUT_SCALE = MAX_PERTURBATION / 128.0  # == 1.0
PAD_SENTINEL = -3.0e38

_CACHE = {}


def _build():
    f32 = mybir.dt.float32
    mult = mybir.AluOpType.mult
    add = mybir.AluOpType.add

    nc = bacc.Bacc(
        "TRN2", target_bir_lowering=False, debug=False, num_devices=1
    )
    # host-padded, partition-major: xs[c, p, :] with p = sample*32 + block
    xs = nc.dram_tensor("xs", [3, P, FD + 1], f32, kind="ExternalInput")
    # wt cols 0:9, identity cols 9:137 — one DMA
    wid = nc.dram_tensor("wid", [P, 9 + P], f32, kind="ExternalInput")
    gmat = nc.dram_tensor("gmat", [NS, P], f32, kind="ExternalInput")
    out = nc.dram_tensor("out", [3, P, FD], f32, kind="ExternalOutput")

    with tile.TileContext(nc) as tc:
        with (
            tc.tile_pool(name="wp", bufs=1) as wp,
            tc.tile_pool(name="xp", bufs=1) as xp,
            tc.tile_pool(name="qp", bufs=2) as qp,
            tc.tile_pool(name="lp", bufs=3) as lp,
            tc.tile_pool(name="st", bufs=2) as st,
            tc.tile_pool(name="pp", bufs=2, space="PSUM") as pp,
            tc.tile_pool(name="op", bufs=3) as outp,
        ):
            # x channel tiles, (128, FD+1); pad col comes zeroed from host.
            # Issue across both HWDGE rings: Sync gets x0/x1, Scalar the rest.
            HF = FD // 2
            xts = []
            for c in range(3):
                xt = xp.tile([P, FD + 1], f32, tag=f"x{c}")
                xts.append(xt)
            x0, x1, x2 = xts
            nc.sync.dma_start(x0[:], xs[0])
            nc.sync.dma_start(x1[:], xs[1])
            widt = wp.tile([P, 9 + P], f32)
            nc.scalar.dma_start(widt[:], wid[:])
            nc.scalar.dma_start(x2[:], xs[2])
            g_t = wp.tile([NS, P], f32)
            nc.scalar.dma_start(g_t[:], gmat[:])
            wt = widt[:, 0:9]
            ident_t = widt[:, 9 : 9 + P]

            for j in range(3):
                wc = lambda c: wt[:, 3 * j + c : 3 * j + c + 1]  # noqa: E731
                # q = x1*w1 + x0*w0 over [0:FD]; pad col = sentinel
                q = qp.tile([P, FD + 1], f32, tag="q")
                nc.vector.memset(q[:, FD : FD + 1], PAD_SENTINEL)
                nc.vector._custom_dve(
                    MULADD2, out=q[:, 0:FD], in0=x1[:, 0:FD],
                    in1=x0[:, 0:FD], s0=wc(1), s1=wc(0),
                )
                # lin = x2*w2 + q; pad col -> running max; accum col -> min
                lin = lp.tile([P, FD + 2], f32, tag="lin")
                nc.vector._custom_dve(
                    LINSTAT, out=lin[:, 0 : FD + 1], in0=x2[:, 0 : FD + 1],
                    in1=q[:, 0 : FD + 1], s0=wc(2), s1=-1.0e38, imm2=3.4e38,
                    accum_out=lin[:, FD + 1 : FD + 2],
                )
                # stats chain at high priority: its tiny ops must interleave
                # between the 2us DVE ops instead of queueing after them
                import contextlib
                with contextlib.nullcontext():
                    # negate min so one grouped max-reduce covers both rows
                    nc.vector.tensor_scalar_mul(
                        lin[:, FD + 1 : FD + 2], lin[:, FD + 1 : FD + 2], -1.0
                    )
                    # stats: (128,2) -> T -> (2,128) -> group max -> (2,4)
                    ps1 = pp.tile([2, P], f32, tag="ps1")
                    nc.tensor.transpose(ps1[:], lin[:, FD : FD + 2], ident_t[:])
                    st4 = st.tile([2, NS], f32, tag="st4")
                    nc.vector.tensor_reduce(
                        st4[:], ps1[:].rearrange("r (n g) -> r n g", g=G32),
                        axis=mybir.AxisListType.X, op=mybir.AluOpType.max,
                    )
                    # (2,4) -> T -> (4,2): r2[:,0]=max, r2[:,1]=-min per sample
                    ps2 = pp.tile([NS, 2], f32, tag="ps2")
                    nc.tensor.transpose(ps2[:], st4[:], ident_t[:2, :2])
                    r2 = st.tile([NS, 2], f32, tag="r2")
                    nc.vector.tensor_copy(r2[:], ps2[:])
                    # s = 2*OUT_SCALE/(max-min);  t = -min*s - OUT_SCALE
                    rngc = st.tile([NS, 1], f32, tag="rngc")
                    nc.vector.tensor_add(rngc[:], r2[:, 0:1], r2[:, 1:2])
                    rcp = st.tile([NS, 1], f32, tag="rcp")
                    nc.vector.reciprocal(rcp[:], rngc[:])
                    st2 = st.tile([NS, 2], f32, tag="st2")
                    nc.vector.tensor_scalar_mul(
                        st2[:, 0:1], rcp[:], 2.0 * OUT_SCALE
                    )
                    nc.vector.tensor_scalar(
                        st2[:, 1:2], r2[:, 1:2], st2[:, 0:1], -OUT_SCALE,
                        op0=mult, op1=add,
                    )
                    # broadcast per-sample [s|t] to all 128 partitions
                    ps3 = pp.tile([P, 2], f32, tag="ps3")
                    nc.tensor.matmul(
                        ps3[:], g_t[:], st2[:], start=True, stop=True
                    )
                    stsb = st.tile([P, 2], f32, tag="stsb")
                    nc.scalar.copy(stsb[:], ps3[:])
                # out = lin*s + t on ACT; DMA issue on the ACT HWDGE ring
                ot = outp.tile([P, FD], f32, tag="ot")
                nc.scalar.activation(
                    ot[:], lin[:, 0:FD],
                    mybir.ActivationFunctionType.Identity,
                    bias=stsb[:, 1:2], scale=stsb[:, 0:1],
                )
                nc.sync.dma_start(out[j], ot[:])

    nc.compile()
    return nc


def get_nc():
    if "nc" not in _CACHE:
        _CACHE["nc"] = _build()
    return _CACHE["nc"]


def make_in_maps(x, target, W, b):
    x = np.ascontiguousarray(np.asarray(x), dtype=np.float32)
    tgt = np.asarray(target).astype(np.int64)
    Wm = np.asarray(W, dtype=np.float32).reshape(20 * OUT_CHANNELS, 3)
    Wsel = Wm.reshape(20, OUT_CHANNELS, 3)[tgt]  # (N, 3 out, 3 in)

    eye = np.eye(P, dtype=np.float32)
    gmat = np.repeat(np.eye(NS, dtype=np.float32), G32, axis=1)
    in_maps = []
    xpad = np.zeros((N, 3, G32, FD + 1), dtype=np.float32)
    xpad[:, :, :, :FD] = x.reshape(N, 3, G32, FD)
    for core in range(NCORES):
        lo = core * NS
        # (c, n, g, f) -> (c, n*32+g, f)
        xs = np.ascontiguousarray(
            xpad[lo : lo + NS].transpose(1, 0, 2, 3).reshape(3, P, FD + 1)
        )
        wcols = np.repeat(
            Wsel[lo : lo + NS].reshape(NS, 1, 9), G32, axis=1
        ).reshape(P, 9)
        wid = np.ascontiguousarray(np.concatenate([wcols, eye], axis=1))
        in_maps.append({"xs": xs, "wid": wid, "gmat": gmat})
    return in_maps


def run(x, target, W, b, trace=False, retries=2):
    nc = get_nc()
    in_maps = make_in_maps(x, target, W, b)
    last_err = None
    for attempt in range(retries + 1):
        try:
            res = run_bass_kernel_spmd(
                nc, in_maps, list(range(NCORES)), trace=trace
            )
            outs = []
            for r in res.results:
                o = r["out"].reshape(3, NS, G32, FD).transpose(1, 0, 2, 3)
                outs.append(o.reshape(NS, OUT_CHANNELS, H, W_))
            return np.concatenate(outs, axis=0), res
        except Exception as e:  # device may need recovery; retry
            last_err = e
            if attempt < retries:
                time.sleep(20)
    raise last_err


def kernel(x, target, W, b):
    out, _ = run(x, target, W, b)
    return out

